# revision 42
# baseline (speedup 1.0000x reference)
"""Devoxelization (trilinear interpolation of voxel features at point
locations) on 8 Trainium2 NeuronCores, data-parallel over the batch.

  pts:  [8, 3, 65536] f32, feat: [8, 64, 32, 32, 32] f32
  out:  [8, 64, 65536] f32

Per core (one batch sample):
  - Host precomputes, exactly mirroring the reference's fp32 math:
    voxel coords, the 4 (x,y)-corner flat indices (z-pair base), and the
    5 per-point scalars (vz, and the 4 bilinear xy corner weights).
  - The feature volume is shipped as a [32768, 128] table whose row v is
    [feat_row(v) | feat_row(v+1) - feat_row(v)]  (values + z-diff), so one
    256B-aligned dma_gather row fetches both z corners of one xy corner.
  - Device: dma_gather rows to SBUF in point-on-partition layout, then per
    point-row: z-lerp via one scalar_tensor_tensor (t = d*vz + g), then the
    weighted xy-corner sum via a tensor_scalar + 3 scalar_tensor_tensor
    chain, all with per-partition scalar weights.
  - Output [N, 64] per core; host casts/transposes to [64, N].

The z 'lerp' uses the reference's non-fractional weights: t = g_l + vz*d is
algebraically equal to g_l*(1-vz) + g_r*vz with g_r = feat[zl+1]. When
ceil(vz)==floor(vz) the reference uses g_r == g_l; the host then sets the
vz scalar to 0 so t == g_l exactly.
"""

import numpy as np
import ml_dtypes

B = 8
C = 64
N = 65536
R = 32
NV = R * R * R  # 32768
EPS = 1e-08

# --- tunables -------------------------------------------------------------
USE_BF16 = True          # table/arith dtype on device; False -> float32
CHUNKS = 512 if USE_BF16 else 512
PTS_PER_PART = N // 128          # 512 points per partition
RB = PTS_PER_PART // CHUNKS      # point-rows per chunk (per partition)
ROWS = 4 * RB                    # gathered rows per chunk (4 xy corners)
NUM_IDXS = ROWS * 128            # gather indices per chunk
IDX_COLS = NUM_IDXS // 16        # wrapped idx columns per chunk

_bf16 = ml_dtypes.bfloat16

_CACHE = {}


def _host_prepare(pts, feat):
    """Replicate the reference's fp32 index/weight math and build the three
    device inputs per batch sample."""
    f32 = np.float32
    pts = np.asarray(pts, dtype=f32)
    feat = np.asarray(feat, dtype=f32)

    p = pts - pts.min(axis=2, keepdims=True)                       # [B,3,N]
    norms = np.sqrt((p * p).sum(axis=1, dtype=f32), dtype=f32)     # [B,N]
    denom = f32(norms.max() + f32(EPS))
    vox = (p / denom) * f32(R - 1)                                 # [B,3,N]
    il = np.floor(vox).astype(np.int32)
    ir = np.ceil(vox).astype(np.int32)

    vx, vy, vz = vox[:, 0], vox[:, 1], vox[:, 2]
    xl, yl, zl = il[:, 0], il[:, 1], il[:, 2]
    xr, yr = ir[:, 0], ir[:, 1]
    vz_eff = np.where(il[:, 2] == ir[:, 2], f32(0.0), vz).astype(f32)

    wxl = (f32(1.0) - vx).astype(f32)
    wxr = vx.astype(f32)
    wyl = (f32(1.0) - vy).astype(f32)
    wyr = vy.astype(f32)

    # corner order k: (xl,yl) (xl,yr) (xr,yl) (xr,yr); all at z-pair base zl
    vmat = np.stack(
        [
            xl * (R * R) + yl * R + zl,
            xl * (R * R) + yr * R + zl,
            xr * (R * R) + yl * R + zl,
            xr * (R * R) + yr * R + zl,
        ],
        axis=1,
    )                                                              # [B,4,N]
    assert vmat.min() >= 0 and vmat.max() <= NV - 2, (vmat.min(), vmat.max())
    vmat = vmat.astype(np.int16)

    w5 = np.stack(
        [vz_eff, wxl * wyl, wxl * wyr, wxr * wyl, wxr * wyr], axis=1
    ).astype(f32)                                                  # [B,5,N]

    dt = _bf16 if USE_BF16 else f32

    in_maps = []
    for b in range(B):
        tab = np.ascontiguousarray(feat[b].reshape(C, NV).T)       # [NV, 64]
        table = np.empty((NV, 2 * C), dtype=f32)
        table[:, :C] = tab
        table[:-1, C:] = tab[1:] - tab[:-1]
        table[-1, C:] = 0.0
        table = np.ascontiguousarray(table.astype(dt))

        # point id n = p*512 + c*RB + rb
        V = vmat[b].reshape(4, 128, CHUNKS, RB)                    # [k,p,c,rb]
        arr = V.transpose(2, 3, 0, 1).reshape(CHUNKS, ROWS * 128)  # [c,(rb,k,p)]
        wrapped = arr.reshape(CHUNKS, IDX_COLS, 16)                # j = s*16+q
        idxs = np.ascontiguousarray(
            np.tile(wrapped.transpose(0, 2, 1), (1, 8, 1))         # [c,128,cols]
            .transpose(1, 0, 2)
            .reshape(128, CHUNKS * IDX_COLS)
        )

        W = w5[b].reshape(5, 128, CHUNKS, RB)
        wts = np.ascontiguousarray(
            W.transpose(1, 2, 3, 0).reshape(128, CHUNKS * RB * 5)
        )

        in_maps.append({"table": table, "idxs": idxs, "wts": wts})
    return in_maps


def _build_program():
    import concourse.bass as bass
    import concourse.bacc as bacc
    import concourse.mybir as mybir
    from concourse.tile import TileContext, add_dep_helper

    dt = mybir.dt.bfloat16 if USE_BF16 else mybir.dt.float32
    MUL = mybir.AluOpType.mult
    ADD = mybir.AluOpType.add

    # HW empirics: one dma_gather tops out near 57 descriptors per side
    # (~896 idxs; DMA packet ceiling); 512 idxs (33+33 descs) is the largest
    # size that keeps a point's 4 corner rows in one gather.
    nc = bacc.Bacc("TRN2", debug=False)
    table = nc.dram_tensor("table", [NV, 2 * C], dt, kind="ExternalInput")
    idxs = nc.dram_tensor(
        "idxs", [128, CHUNKS * IDX_COLS], mybir.dt.int16, kind="ExternalInput"
    )
    wts = nc.dram_tensor(
        "wts", [128, CHUNKS * RB * 5], mybir.dt.float32, kind="ExternalInput"
    )
    out = nc.dram_tensor("out", [128, CHUNKS * RB * C], dt, kind="ExternalOutput")

    GRP = 128  # chunks per output DMA (keeps total HWDGE DMA count <= 8)

    with TileContext(nc) as tc:
        with (
            tc.tile_pool(name="wp", bufs=1) as wp,
            tc.tile_pool(name="ip", bufs=1) as ip,
            tc.tile_pool(name="gp", bufs=8) as gp,
            tc.tile_pool(name="tp", bufs=4) as tp,
            tc.tile_pool(name="mp", bufs=4) as mp,
            tc.tile_pool(name="op", bufs=2) as op,
            tc.tile_pool(name="pp", bufs=CHUNKS) as pp,
        ):
            wt = wp.tile([128, CHUNKS * RB * 5], mybir.dt.float32)
            hw_dmas = [nc.sync.dma_start(wt[:, :], wts[:, :])]
            it = ip.tile([128, CHUNKS * IDX_COLS], mybir.dt.int16)
            hw_dmas.append(nc.sync.dma_start(it[:, :], idxs[:, :]))
            # sink absorbs DMA-completion sem waits on a plain copy so the
            # STT instructions (few sync-wait slots) rely on same-engine
            # ordering instead.
            sink = wp.tile([128, 1], mybir.dt.float32)
            nc.vector.tensor_copy(sink[:, :], wt[:, 0:1])
            psink = wp.tile([128, 1], mybir.dt.int16)
            nc.gpsimd.tensor_copy(psink[:, :], it[:, 0:1])
            psb = wp.tile([128, CHUNKS], dt)

            # walrus allows a single sync-wait per instruction, so every
            # instruction that would need 2+ waits gets preceding absorber
            # ops (1 wait each); later ops ride same-engine ordering.
            gathers = []
            ot = None
            for c in range(CHUNKS):
                g = gp.tile([128, ROWS, 2 * C], dt)
                if c >= 1 and (c % 4 == 1 or c < 8):
                    # Pool observes the previous gather's DMA completion; by
                    # induction its clock then covers every earlier DMASW
                    # lane (slot WAW distance is 8, every 4th chunk is
                    # enough), so memset/gather waits stay at <= 1.
                    x = nc.gpsimd.memset(psb[:, c : c + 1], 0)
                    add_dep_helper(
                        x.ins, gathers[c - 1].ins, sync=True,
                        reason="pool observes prev gather dma",
                    )
                # The psb dep-chain keeps Pool's clock over the DMASW lanes,
                # so the gather's only sem wait is the slot's DVE release.
                gi = nc.gpsimd.dma_gather(
                    g[:, :, :],
                    table[:, :],
                    it[:, c * IDX_COLS : (c + 1) * IDX_COLS],
                    NUM_IDXS,
                    NUM_IDXS,
                    2 * C,
                    single_packet=False,
                )
                gathers.append(gi)
                if c % GRP == 0:
                    ot = op.tile([128, GRP * RB * C], dt)
                    nc.vector.tensor_copy(ot[:, 0:1], wt[:, 0:1])
                obase = (c % GRP) * RB * C
                sinkc = wp.tile([128, 1], mybir.dt.float32)
                nc.vector.tensor_copy(sinkc[:, :], g[:, 1, 0:1])
                for rb in range(RB):
                    wcol = lambda s: wt[
                        :, c * RB * 5 + rb * 5 + s : c * RB * 5 + rb * 5 + s + 1
                    ]
                    t = tp.tile([128, 4, C], dt)
                    # z-lerp for all 4 xy corners: t = d*vz + g_l
                    nc.vector.scalar_tensor_tensor(
                        t[:, :, :],
                        g[:, 4 * rb : 4 * rb + 4, C : 2 * C],
                        wcol(0),
                        g[:, 4 * rb : 4 * rb + 4, 0:C],
                        MUL,
                        ADD,
                    )
                    m0 = mp.tile([128, C], dt)
                    nc.scalar.mul(m0[:, :], t[:, 0, :], wcol(1))
                    m1 = mp.tile([128, C], dt)
                    nc.vector.scalar_tensor_tensor(
                        m1[:, :], t[:, 1, :], wcol(2), m0[:, :], MUL, ADD
                    )
                    m2 = mp.tile([128, C], dt)
                    nc.vector.scalar_tensor_tensor(
                        m2[:, :], t[:, 2, :], wcol(3), m1[:, :], MUL, ADD
                    )
                    last_dve = nc.vector.scalar_tensor_tensor(
                        ot[:, obase + rb * C : obase + (rb + 1) * C],
                        t[:, 3, :],
                        wcol(4),
                        m2[:, :],
                        MUL,
                        ADD,
                    )
                if c % GRP == GRP - 1:
                    gbase = (c - GRP + 1) * RB * C
                    hw_dmas.append(
                        nc.sync.dma_start(
                            out[:, gbase : gbase + GRP * RB * C], ot[:, :]
                        )
                    )

            # Pre-absorb the kernel-tail drain's sem waits: one SP nop per
            # proc the drain would otherwise wait on (the drain's CTRL
            # struct holds very few sync waits).
            last_pool = nc.gpsimd.memset(psb[:, 0:1], 0)
            for ref in gathers[-8:] + hw_dmas + [last_pool, last_dve]:
                nop = nc.sync.nop(nofuse=True)
                add_dep_helper(
                    nop.ins, ref.ins, sync=True, reason="tail drain pre-absorb"
                )
    nc.compile()
    return nc


def kernel(pts, feat):
    from concourse import bass_utils

    in_maps = _host_prepare(pts, feat)

    if "nc" not in _CACHE:
        _CACHE["nc"] = _build_program()
    nc = _CACHE["nc"]

    res = bass_utils.run_bass_kernel_spmd(nc, in_maps, core_ids=list(range(B)))
    global LAST_EXEC_NS
    LAST_EXEC_NS = getattr(res, "exec_time_ns", None)

    out = np.empty((B, C, N), dtype=np.float32)
    for b in range(B):
        o = np.asarray(res.results[b]["out"])
        # [128, CHUNKS*RB*C] -> [N, C] (point id n = p*512 + c*RB + rb) -> [C, N]
        out[b] = o.astype(np.float32).reshape(N, C).T
    return out
